# revision 1
# baseline (speedup 1.0000x reference)
"""Causal self-attention (B=2, S=2048, E=2048, H=16) on 8 TRN2 NeuronCores.

Sharding: core = 4*b + hg  (b in {0,1} data-parallel over batch,
hg in {0..3} tensor-parallel over groups of 4 heads / Wqkv columns /
Wo rows).  Each core computes a partial [S, E] output (its heads'
contribution through Wo); the host sums the 4 partials per batch.

Kernel math (per core, heads h=0..3 local):
  qT/kT [D=128, S] = Wq/Wk_cols.T @ x[b].T    (xT host-transposed)
  RoPE applied in [D, S] layout: rot = q*cos + (Sw @ q)*sin, where Sw is
  the pair-swap/sign permutation done as a 128x128 matmul; 1/sqrt(D) is
  folded into Wq on the host.
  v [S, 512] natural layout (all 4 heads), accumulated 4 s-chunks at a
  time with the e-loop outermost so the PE has work while xT streams in.
  scoresT [k, q] = kT_c.T @ qT_w  (D contraction, one matmul per block,
  causal blocks only, 4 host-built mask tiles for the diagonal).
  expP = exp(scoresT) on ACT; denom = allones.T @ expP (PE, result
  broadcast across partitions); oT [D, q] = sum_c v_c-as-lhsT @ expP;
  oT *= recip(denom) (DVE).
  out_partial [S, E] = sum_h oT_h.T-slices @ Wo_rows (PE), DMA to DRAM.

All matmul operands are float32r (TF32-like fast fp32 mode, ~1.2e-4
rounding), accumulation fp32 in PSUM.
"""
import sys

if "/opt/trn_rl_repo" not in sys.path:
    sys.path.insert(0, "/opt/trn_rl_repo")

from contextlib import ExitStack

import numpy as np

import concourse.bass as bass
import concourse.mybir as mybir
import concourse.tile as tile
from concourse import bacc, bass_utils

F32 = mybir.dt.float32
F32R = mybir.dt.float32r
AF = mybir.ActivationFunctionType

B = 2
S = 2048
E = 2048
H = 16
D = 128
HL = 4            # heads per core
P = 128
NE = E // P       # 16 contraction chunks
NW = S // 512     # 4 q windows of 512
NSC = S // P      # 16 s-chunks
NEG = -1.0e30

_PROGRAM = None


def _build_program():
    nc = bacc.Bacc("TRN2", target_bir_lowering=False, debug=False, num_devices=8)

    xT = nc.dram_tensor("xT", [E, S], F32R, kind="ExternalInput").ap()
    wq = nc.dram_tensor("wq", [E, HL * D], F32R, kind="ExternalInput").ap()
    wk = nc.dram_tensor("wk", [E, HL * D], F32R, kind="ExternalInput").ap()
    wv = nc.dram_tensor("wv", [E, HL * D], F32R, kind="ExternalInput").ap()
    wo = nc.dram_tensor("wo", [HL * D, E], F32R, kind="ExternalInput").ap()
    cosf = nc.dram_tensor("cosf", [P, S], F32, kind="ExternalInput").ap()
    sinf = nc.dram_tensor("sinf", [P, S], F32, kind="ExternalInput").ap()
    swapT = nc.dram_tensor("swapT", [P, P], F32R, kind="ExternalInput").ap()
    onesc = nc.dram_tensor("onesc", [P, P], F32R, kind="ExternalInput").ap()
    maskT = nc.dram_tensor("maskT", [P, 4 * 512], F32, kind="ExternalInput").ap()
    outp = nc.dram_tensor("outp", [S, E], F32, kind="ExternalOutput").ap()

    # DRAM scratch (per-core local); per-head q/k spill tensors so a head's
    # reload only depends on that head's spill writes
    qk_sps = [nc.dram_tensor(f"qk_sp{h}", [2 * P, S], F32R).ap() for h in range(HL)]
    v_sp = nc.dram_tensor("v_sp", [P, NSC * HL * D], F32R).ap()   # [p, c*512 + h*128 + d]

    with tile.TileContext(nc) as tc:
        with ExitStack() as ctx_all:
            cpool = ctx_all.enter_context(tc.tile_pool(name="const", bufs=1))
            sw_t = cpool.tile([P, P], F32R, tag="swap")
            nc.sync.dma_start(sw_t[:], swapT)
            ones_t = cpool.tile([P, P], F32R, tag="ones")
            nc.sync.dma_start(ones_t[:], onesc)

            # ---------------- Phase 1: QKV projection + RoPE ----------------
            with ExitStack() as ctx1:
                xpool = ctx1.enter_context(tc.tile_pool(name="xT", bufs=NE))
                wv_pool = ctx1.enter_context(tc.tile_pool(name="wv", bufs=NE))
                vst_pool = ctx1.enter_context(tc.tile_pool(name="vst", bufs=2))
                cspool = ctx1.enter_context(tc.tile_pool(name="cossin", bufs=1))
                wqk_pool = ctx1.enter_context(tc.tile_pool(name="wqk", bufs=2))
                raw_pool = ctx1.enter_context(tc.tile_pool(name="raw", bufs=2))
                tmp_pool = ctx1.enter_context(tc.tile_pool(name="tmp", bufs=1))
                rot_pool = ctx1.enter_context(tc.tile_pool(name="rot", bufs=2))

                xt = []
                wvt = []
                for e in range(NE):
                    tw = wv_pool.tile([P, HL * D], F32R, tag="wv", name=f"wv_{e}")
                    nc.sync.dma_start(tw[:], wv[e * P:(e + 1) * P, :])
                    wvt.append(tw)
                    t = xpool.tile([P, S], F32R, tag="x", name=f"x_{e}")
                    nc.sync.dma_start(t[:], xT[e * P:(e + 1) * P, :])
                    xt.append(t)
                cos_t = cspool.tile([P, S], F32, tag="cos")
                nc.sync.dma_start(cos_t[:], cosf)
                sin_t = cspool.tile([P, S], F32, tag="sin")
                nc.sync.dma_start(sin_t[:], sinf)

                # v: 2 passes of 8 s-chunks, e-loop outermost (PE has 8
                # matmuls ready per arriving xT tile during the load)
                with ExitStack() as ctxv:
                    ps_v = ctxv.enter_context(tc.tile_pool(name="ps_v", bufs=8, space="PSUM"))
                    for g in range(2):
                        psvs = [ps_v.tile([P, 512], F32, tag="v", name=f"psv_{g}_{ci}") for ci in range(8)]
                        for e in range(NE):
                            for ci in range(8):
                                c = g * 8 + ci
                                nc.tensor.matmul(psvs[ci][:], xt[e][:, c * P:(c + 1) * P],
                                                 wvt[e][:],
                                                 start=(e == 0), stop=(e == NE - 1))
                        for ci in range(8):
                            c = g * 8 + ci
                            vst = vst_pool.tile([P, 512], F32R, tag="vst")
                            nc.vector.tensor_copy(vst[:], psvs[ci][:])
                            nc.sync.dma_start(v_sp[:, c * 512:(c + 1) * 512], vst[:])

                # q, k with RoPE, per head
                ps_qk = ctx1.enter_context(tc.tile_pool(name="ps_qk", bufs=2, space="PSUM"))
                ps_sw = ctx1.enter_context(tc.tile_pool(name="ps_sw", bufs=2, space="PSUM"))
                for h in range(HL):
                    for m, wsrc in ((0, wq), (1, wk)):
                        wt = wqk_pool.tile([P, NE, P], F32R, tag="wqk")
                        nc.sync.dma_start(
                            wt[:],
                            wsrc[:, h * D:(h + 1) * D].rearrange("(n p) c -> p n c", p=P))
                        for w in range(NW):
                            ws = slice(w * 512, (w + 1) * 512)
                            ps = ps_qk.tile([P, 512], F32, tag="qk")
                            for e in range(NE):
                                nc.tensor.matmul(ps[:], wt[:, e, :], xt[e][:, ws],
                                                 start=(e == 0), stop=(e == NE - 1))
                            raw = raw_pool.tile([P, 512], F32R, tag="raw")
                            nc.vector.tensor_copy(raw[:], ps[:])
                            pssw = ps_sw.tile([P, 512], F32, tag="sw")
                            nc.tensor.matmul(pssw[:], sw_t[:], raw[:],
                                             start=True, stop=True)
                            tmp = tmp_pool.tile([P, 512], F32, tag="tmp")
                            nc.vector.tensor_mul(tmp[:], raw[:].bitcast(F32), cos_t[:, ws])
                            # sin product written in place into the psum tile
                            nc.vector.tensor_mul(pssw[:], pssw[:], sin_t[:, ws])
                            rot = rot_pool.tile([P, 512], F32R, tag="rot")
                            nc.vector.tensor_add(rot[:], tmp[:], pssw[:])
                            nc.sync.dma_start(qk_sps[h][m * P:(m + 1) * P, ws], rot[:])

            # ---------------- Phase 2+3: attention, then Wo ----------------
            with ExitStack() as ctx2:
                qk_pool = ctx2.enter_context(tc.tile_pool(name="qk_re", bufs=6))
                v2_pool = ctx2.enter_context(tc.tile_pool(name="v_re", bufs=2))
                mpool = ctx2.enter_context(tc.tile_pool(name="mask", bufs=1))
                exp_pool = ctx2.enter_context(tc.tile_pool(name="expp", bufs=6))
                rec_pool = ctx2.enter_context(tc.tile_pool(name="rec", bufs=2))
                oT_pool = ctx2.enter_context(tc.tile_pool(name="oT", bufs=HL))
                wo_pool = ctx2.enter_context(tc.tile_pool(name="wo", bufs=HL))
                ost_pool = ctx2.enter_context(tc.tile_pool(name="ost", bufs=4))

                mask_t = mpool.tile([P, 4 * 512], F32, tag="mask")
                nc.sync.dma_start(mask_t[:], maskT)
                wot = []
                oTt = [oT_pool.tile([P, S], F32R, tag="oT", name=f"oT_{hh}") for hh in range(HL)]

                v_sp_r = v_sp.rearrange("p (c g d) -> p c g d", c=NSC, g=HL)

                with ExitStack() as ctx2p:
                    ps_s = ctx2p.enter_context(tc.tile_pool(name="ps_s", bufs=4, space="PSUM"))
                    ps_o = ctx2p.enter_context(tc.tile_pool(name="ps_o", bufs=2, space="PSUM"))
                    ps_d = ctx2p.enter_context(tc.tile_pool(name="ps_d", bufs=2, space="PSUM"))

                    for h in range(HL):
                        qr = qk_pool.tile([P, S], F32R, tag="qk")
                        kr = qk_pool.tile([P, S], F32R, tag="qk")
                        vh = v2_pool.tile([P, NSC, D], F32R, tag="v")
                        for w0 in range(NW):
                            s0 = slice(w0 * 512, (w0 + 1) * 512)
                            nc.sync.dma_start(kr[:, s0], qk_sps[h][P:2 * P, s0])
                            nc.sync.dma_start(qr[:, s0], qk_sps[h][0:P, s0])
                            nc.sync.dma_start(vh[:, w0 * 4:(w0 + 1) * 4, :],
                                              v_sp_r[:, w0 * 4:(w0 + 1) * 4, h, :])
                        if h == 0:
                            # prefetch Wo now (after head-0 operand reloads)
                            for h2 in range(HL):
                                t = wo_pool.tile([P, E], F32R, tag="wo", name=f"wo_{h2}")
                                nc.sync.dma_start(t[:], wo[h2 * P:(h2 + 1) * P, :])
                                wot.append(t)

                        for w in range(NW):
                            ws = slice(w * 512, (w + 1) * 512)
                            nblk = 4 * w + 4
                            pso = ps_o.tile([P, 512], F32, tag="o")
                            psd = ps_d.tile([P, 512], F32, tag="d")
                            for c in range(nblk):
                                r = c - 4 * w
                                # valid q-columns of this block start at 128r;
                                # matmuls clamp to >=256 wide for full f32r rate
                                ex0 = max(0, 128 * r)        # first col exp writes
                                mm0 = min(ex0, 256)          # first col matmuls touch
                                mms = slice(w * 512 + mm0, (w + 1) * 512)
                                pss = ps_s.tile([P, 512], F32, tag="s")
                                nc.tensor.matmul(pss[:, mm0:], kr[:, c * P:(c + 1) * P],
                                                 qr[:, mms], start=True, stop=True)
                                if r >= 0 and ex0 < 512:
                                    nc.vector.tensor_add(
                                        pss[:, ex0:ex0 + P], pss[:, ex0:ex0 + P],
                                        mask_t[:, r * 512 + ex0:r * 512 + ex0 + P])
                                expc = exp_pool.tile([P, 512], F32R, tag="e")
                                if ex0 > 0:
                                    nc.gpsimd.memset(expc[:, 0:ex0].bitcast(F32), 0.0)
                                nc.scalar.activation(expc[:, ex0:], pss[:, ex0:], AF.Exp)
                                nc.tensor.matmul(pso[:, mm0:], vh[:, c, :], expc[:, mm0:],
                                                 start=(c == 0), stop=(c == nblk - 1))
                                nc.tensor.matmul(psd[:, mm0:], ones_t[:], expc[:, mm0:],
                                                 start=(c == 0), stop=(c == nblk - 1))
                            recd = rec_pool.tile([P, 512], F32, tag="r")
                            nc.vector.reciprocal_approx_fast(out=recd[:], in_=psd[:])
                            nc.vector.tensor_mul(oTt[h][:, ws], pso[:], recd[:])

                with ExitStack() as ctx3p:
                    ps_out = ctx3p.enter_context(tc.tile_pool(name="ps_out", bufs=4, space="PSUM"))
                    for sc in range(NSC):
                        for w in range(NW):
                            ws = slice(w * 512, (w + 1) * 512)
                            pso = ps_out.tile([P, 512], F32, tag="po")
                            for h in range(HL):
                                nc.tensor.matmul(pso[:], oTt[h][:, sc * P:(sc + 1) * P],
                                                 wot[h][:, ws],
                                                 start=(h == 0), stop=(h == HL - 1))
                            ost = ost_pool.tile([P, 512], F32, tag="ost")
                            nc.vector.tensor_copy(ost[:], pso[:])
                            nc.sync.dma_start(outp[sc * P:(sc + 1) * P, ws], ost[:])

    nc.compile()
    return nc


def _get_program():
    global _PROGRAM
    if _PROGRAM is None:
        _PROGRAM = _build_program()
    return _PROGRAM


def _host_prep(x, Wqkv, Wo, freqs_cis):
    """Build the 8 per-core input maps."""
    x = np.asarray(x, dtype=np.float32)
    Wqkv = np.asarray(Wqkv, dtype=np.float32)
    Wo = np.asarray(Wo, dtype=np.float32)
    freqs_cis = np.asarray(freqs_cis, dtype=np.float32)

    scale = np.float32(D ** -0.5)
    cos = freqs_cis[:, 0, :, 0].T        # [64, S]
    sin = freqs_cis[:, 0, :, 1].T
    cosf = np.ascontiguousarray(np.repeat(cos, 2, axis=0).astype(np.float32))
    sinf = np.ascontiguousarray(np.repeat(sin, 2, axis=0).astype(np.float32))

    # SwapSign @ q: out[2i] = -q[2i+1], out[2i+1] = q[2i]; lhsT = SwapSign.T
    sw = np.zeros((P, P), dtype=np.float32)
    ii = np.arange(0, P, 2)
    sw[ii, ii + 1] = -1.0
    sw[ii + 1, ii] = 1.0
    swapT = np.ascontiguousarray(sw.T)

    onesc = np.ones((P, P), dtype=np.float32)

    # mask tiles: maskT[k, r*512 + q] = 0 if k + 128r <= q else -1e30
    kk = np.arange(P)[:, None]
    qq = np.arange(512)[None, :]
    maskT = np.concatenate(
        [np.where(kk + P * r <= qq, 0.0, NEG).astype(np.float32) for r in range(4)],
        axis=1)
    maskT = np.ascontiguousarray(maskT)

    in_maps = []
    for core in range(8):
        b, hg = divmod(core, 4)
        cs = slice(hg * 512, (hg + 1) * 512)
        in_maps.append({
            "xT": np.ascontiguousarray(x[b].T),
            "wq": np.ascontiguousarray(Wqkv[:, 0 * E:1 * E][:, cs]) * scale,
            "wk": np.ascontiguousarray(Wqkv[:, 1 * E:2 * E][:, cs]),
            "wv": np.ascontiguousarray(Wqkv[:, 2 * E:3 * E][:, cs]),
            "wo": np.ascontiguousarray(Wo[hg * 512:(hg + 1) * 512, :]),
            "cosf": cosf,
            "sinf": sinf,
            "swapT": swapT,
            "onesc": onesc,
            "maskT": maskT,
        })
    return in_maps


def run_cores(x, Wqkv, Wo, freqs_cis, trace=False, **kw):
    """Run the 8-core SPMD program; returns (partials list, BassKernelResults)."""
    nc = _get_program()
    in_maps = _host_prep(x, Wqkv, Wo, freqs_cis)
    res = bass_utils.run_bass_kernel_spmd(
        nc, in_maps, core_ids=list(range(8)), trace=trace, **kw)
    return [r["outp"] for r in res.results], res


def kernel(x, Wqkv, Wo, freqs_cis):
    partials, _ = run_cores(x, Wqkv, Wo, freqs_cis)
    out = np.empty((B, S, E), dtype=np.float32)
    for b in range(B):
        acc = partials[4 * b].astype(np.float32)
        for hg in range(1, 4):
            acc = acc + partials[4 * b + hg]
        out[b] = acc
    return out

